# revision 29
# baseline (speedup 1.0000x reference)
"""BEVTextCLIPLoss Trainium2 kernel.

Sharding: data-parallel over batch B=16 across 8 NeuronCores (2 batches/core).
The local contrastive loss is embarrassingly parallel over B; each core returns
partial sums which the host combines. The tiny global/cross CLIP CE losses are
computed redundantly (fp32) on every core from one packed input.

Layout for the big [B,256,64,64] features: natural C-on-partition layout
([128c x 4096tok] chunks, contiguous HBM rows), cast f32->bf16 during the
SWDGE DMA.  Per-token contractions over C (sum of squares, img.bev, pt.bev,
and the row dots against bev_sum) are elementwise products (DVE/ACT) reduced
across partitions by TensorE matvecs against an all-ones [128,32] stationary
operand, written as 32-replicated rows at PSUM partition groups 0/32/64/96
(PE outputs must be 32-aligned).  Each PSUM generation is staged to SBUF with
one wide ACT copy, and each needed row is DMA-shuffled into its own [128,32]
tile so row-domain math runs at full 128-lane width.  bev_sum comes from a
fused multiply+free-axis-reduce against inv_norm(bev) broadcast across
partitions with a K=1 TensorE matmul into PSUM.  All transcendentals use only
the natural_log_exp ACT table set (rsqrt = exp(-0.5 ln x),
sigmoid = 1/(1+exp(-x))) to avoid table reloads.

Hardware constraint honored throughout: compute instructions support at most
~2 sync waits, and every DMA lands on one of 8 rotating queue semaphores —
so no tile consumed by a single compute op is ever assembled by more than
one or two DMAs.
"""

import math
from contextlib import ExitStack
from functools import lru_cache

import numpy as np

import concourse.bacc as bacc
import concourse.bass as bass
import concourse.mybir as mybir
import concourse.tile as tile
from concourse.alu_op_type import AluOpType
from concourse.bass_utils import run_bass_kernel_spmd

dt = mybir.dt
AFT = mybir.ActivationFunctionType
AXL = mybir.AxisListType

N_CORES = 8
B_FULL = 16
CG = 512
BPC = B_FULL // N_CORES  # batches per core
C = 256
NTOK = 64 * 64  # 4096
NCH = C // 128  # 2 chunks of 128 channels
EPS = 1e-8
GPACK = 3 * CG + 16 + 1  # packed [img | pt | txt | idn16 | logit_scale]


def _build_kernel(ctx, tc, fimg, fpt, fbev, gall, out_d):
    nc = tc.nc
    f32, bf16 = dt.float32, dt.bfloat16
    ones_f32 = nc.const_aps.tensor(1.0, (128, 1), f32)

    sm = ctx.enter_context(tc.tile_pool(name="sm", bufs=2))
    acc = ctx.enter_context(tc.tile_pool(name="acc", bufs=1))

    # accumulators: cols 0..3 = pos diag sums (b0 img, b0 pt, b1 img, b1 pt),
    # cols 4..5 = neg sigmoid sums per batch
    pn_acc = acc.tile([128, 6], f32)
    nc.gpsimd.memset(pn_acc[:], 0.0)

    # stationary operands: all-ones [128,32] (bf16) for the column-sum
    # matvecs, and an all-ones [1,128] row for K=1 partition broadcasts
    ones32 = acc.tile([128, 32], bf16)
    nc.gpsimd.memset(ones32[:], 1.0)
    ones_row = acc.tile([1, 128], bf16)
    nc.gpsimd.memset(ones_row[:], 1.0)

    out_sb = acc.tile([1, 4], f32)

    # ------------------------------------------------------------------
    # Global + cross-modal CLIP CE losses (fp32, tiny, replicated per core)
    # ------------------------------------------------------------------
    with tc.tile_pool(name="glob", bufs=2) as gp, tc.tile_pool(
        name="pg", bufs=2, space="PSUM"
    ) as pg:
        g = gp.tile([16, GPACK], f32, tag="g")
        nc.sync.dma_start(g[:], gall[:, :])
        g3 = g[:, 0 : 3 * CG].rearrange("p (k c) -> p k c", k=3)
        idn_sb = g[:, 3 * CG : 3 * CG + 16]
        ls16 = g[:, 3 * CG + 16 : 3 * CG + 17]

        # exp(logit_scale), already present on all 16 partitions
        s16 = gp.tile([16, 1], f32, tag="s16")
        nc.scalar.activation(s16[:], ls16, AFT.Exp)

        # per-row L2 norms -> inverse via exp(-0.5*ln(x))
        gsq = gp.tile([16, 3], f32, tag="gsq")
        junk = gp.tile([16, 3, CG], f32, tag="gjunk")
        nc.vector.tensor_mul(junk[:], g3, g3)
        nc.vector.tensor_reduce(gsq[:], junk[:], AXL.X, AluOpType.add)
        glog = gp.tile([16, 3], f32, tag="glog")
        nc.scalar.activation(glog[:], gsq[:], AFT.Ln)
        ginv = gp.tile([16, 3], f32, tag="ginv")
        nc.scalar.activation(ginv[:], glog[:], AFT.Exp, scale=-0.5)

        gn = gp.tile([16, 3, CG], f32, tag="gn")
        nc.vector.tensor_mul(
            gn[:], g3, ginv[:].unsqueeze(2).broadcast_to((16, 3, CG))
        )

        # transpose normalized globals to [128c, 16] chunks for the matmuls
        gt = gp.tile([128, 3, 64], f32, tag="gt")
        for k in range(3):
            tp = pg.tile([128, 64], f32, tag="tp")
            for j in range(4):
                nc.tensor.transpose(
                    tp[:, 16 * j : 16 * (j + 1)],
                    gn[:, k, 128 * j : 128 * (j + 1)],
                    idn_sb,
                )
            nc.vector.tensor_copy(gt[:, k, :], tp[:])

        # logit matrices (scaled by exp(logit_scale)):
        # slots 0..4 = [it, it^T, ip, ip^T, pt]
        M = gp.tile([16, 5, 16], f32, tag="M")
        for slot, a, b in ((0, 0, 2), (2, 0, 1), (4, 1, 2)):
            lp = pg.tile([16, 16], f32, tag="lg")
            for j in range(4):
                nc.tensor.matmul(
                    lp[:],
                    gt[:, a, 16 * j : 16 * (j + 1)],
                    gt[:, b, 16 * j : 16 * (j + 1)],
                    start=(j == 0),
                    stop=(j == 3),
                )
            nc.vector.tensor_scalar_mul(M[:, slot, :], lp[:], s16[:])
        for dst, srcslot in ((1, 0), (3, 2)):
            tt = pg.tile([16, 16], f32, tag="lg")
            nc.tensor.transpose(tt[:], M[:, srcslot, :], idn_sb)
            nc.vector.tensor_copy(M[:, dst, :], tt[:])

        # cross-entropy over rows for all 5 matrices at once
        rm = gp.tile([16, 5], f32, tag="rm")
        nc.vector.tensor_reduce(rm[:], M[:], AXL.X, AluOpType.max)
        msh = gp.tile([16, 5, 16], f32, tag="msh")
        nc.vector.tensor_sub(
            msh[:], M[:], rm[:].unsqueeze(2).broadcast_to((16, 5, 16))
        )
        ex = gp.tile([16, 5, 16], f32, tag="ex")
        nc.scalar.activation(ex[:], msh[:], AFT.Exp)
        zz = gp.tile([16, 5], f32, tag="zz")
        nc.vector.tensor_reduce(zz[:], ex[:], AXL.X, AluOpType.add)
        lz = gp.tile([16, 5], f32, tag="lz")
        nc.scalar.activation(lz[:], zz[:], AFT.Ln)
        dg = gp.tile([16, 5, 16], f32, tag="dgm")
        nc.vector.tensor_mul(
            dg[:], msh[:], idn_sb.unsqueeze(1).broadcast_to((16, 5, 16))
        )
        ds = gp.tile([16, 5], f32, tag="ds")
        nc.vector.tensor_reduce(ds[:], dg[:], AXL.X, AluOpType.add)
        lpd = gp.tile([16, 5], f32, tag="lpd")
        nc.vector.tensor_sub(lpd[:], ds[:], lz[:])

        ce_ps = pg.tile([1, 5], f32, tag="fin")
        nc.tensor.matmul(
            ce_ps[:], ones_f32[:16, :], lpd[:], start=True, stop=True
        )
        ce = gp.tile([1, 8], f32, tag="ce")
        nc.vector.tensor_copy(ce[:, 0:5], ce_ps[:])
        # l_global_raw = -(s0+s1+s2+s3)/64 ; l_cross_raw = -(s0+s4)/32
        nc.vector.tensor_add(ce[:, 5:6], ce[:, 0:1], ce[:, 1:2])
        nc.vector.tensor_add(ce[:, 6:7], ce[:, 2:3], ce[:, 3:4])
        nc.vector.tensor_add(ce[:, 5:6], ce[:, 5:6], ce[:, 6:7])
        nc.vector.tensor_scalar_mul(out_sb[:, 2:3], ce[:, 5:6], -1.0 / 64.0)
        nc.vector.tensor_add(ce[:, 7:8], ce[:, 0:1], ce[:, 4:5])
        nc.vector.tensor_scalar_mul(out_sb[:, 3:4], ce[:, 7:8], -1.0 / 32.0)

    # ------------------------------------------------------------------
    # Local contrastive loss over this core's BPC batches
    # ------------------------------------------------------------------
    feat = ctx.enter_context(tc.tile_pool(name="feat", bufs=2))
    prod = ctx.enter_context(tc.tile_pool(name="prod", bufs=3))
    rows = ctx.enter_context(tc.tile_pool(name="rows", bufs=2))
    rsh = ctx.enter_context(tc.tile_pool(name="rsh", bufs=2))
    pctx = ExitStack()
    pp = pctx.enter_context(tc.tile_pool(name="pp", bufs=1, space="PSUM"))

    def row_tile(src, srcpart, dtype=bf16):
        # one [1,4096] row living on partition `srcpart` of src -> [128,32]
        t = rsh.tile([128, 32], dtype, tag=f"row{srcpart}")
        nc.sync.dma_start(
            t[:],
            src[srcpart : srcpart + 1, :].rearrange("a (p j) -> a p j", p=128),
        )
        return t

    def matvec(pt_psum, grp, lhsT, rhs, cch):
        # psum[32g : 32g+32, :] += lhsT.T @ rhs  (32 replicated rows)
        for j in range(8):
            nc.tensor.matmul(
                pt_psum[32 * grp : 32 * (grp + 1), 512 * j : 512 * (j + 1)],
                lhsT,
                rhs[:, 512 * j : 512 * (j + 1)],
                start=(cch == 0),
                stop=(cch == NCH - 1),
                tile_position=(0, 32 * grp),
            )

    def rsqrt_rows(x, tag):
        # exp(-0.5*ln(x)) per [128,32] tile
        t1 = sm.tile([128, 32], f32, tag="lns")
        nc.scalar.activation(t1[:], x[:], AFT.Ln)
        t2 = sm.tile([128, 32], f32, tag=tag)
        nc.scalar.activation(t2[:], t1[:], AFT.Exp, scale=-0.5)
        return t2

    for b in range(BPC):
        I, P, V = [], [], []
        for name, dram, lst in (("I", fimg, I), ("P", fpt, P), ("V", fbev, V)):
            for cch in range(NCH):
                t = feat.tile([128, NTOK], bf16, tag=f"{name}{cch}")
                nc.gpsimd.dma_start(
                    t[:], dram[b, 128 * cch : 128 * (cch + 1), :]
                )
                lst.append(t)

        # generation A: token-wise C-contractions ||I||^2,||P||^2,||V||^2,I.V
        pa = pp.tile([128, NTOK], f32, tag="pgen")
        for cch in range(NCH):
            sqi = prod.tile([128, NTOK], bf16, tag="prod")
            nc.scalar.activation(sqi[:], I[cch][:], AFT.Square)
            matvec(pa, 0, ones32[:], sqi, cch)
            sqp = prod.tile([128, NTOK], bf16, tag="prod")
            nc.scalar.activation(sqp[:], P[cch][:], AFT.Square)
            matvec(pa, 1, ones32[:], sqp, cch)
            sqv = prod.tile([128, NTOK], bf16, tag="prod")
            nc.vector.tensor_mul(sqv[:], V[cch][:], V[cch][:])
            matvec(pa, 2, ones32[:], sqv, cch)
            div = prod.tile([128, NTOK], bf16, tag="prod")
            nc.vector.tensor_mul(div[:], I[cch][:], V[cch][:])
            matvec(pa, 3, ones32[:], div, cch)

        ra = rows.tile([128, NTOK], bf16, tag="ra")
        nc.scalar.copy(ra[:], pa[:])
        sq_i = row_tile(ra, 0)
        sq_p = row_tile(ra, 32)
        sq_v = row_tile(ra, 64)
        d_iv = row_tile(ra, 96)

        # inverse norms: exp(-0.5*ln(sumsq))
        inv_i = rsqrt_rows(sq_i, "inv_i")
        inv_p = rsqrt_rows(sq_p, "inv_p")
        inv_v = rsqrt_rows(sq_v, "inv_v")

        # inv_v -> bf16 row on partition 0 of a scratch, then K=1 matmul
        # broadcast into PSUM [128, NTOK]
        ivbf = sm.tile([128, 32], bf16, tag="ivbf")
        nc.vector.tensor_copy(ivbf[:], inv_v[:])
        ivrow = sm.tile([1, NTOK], bf16, tag="ivrow")
        nc.sync.dma_start(
            ivrow[0:1, :].rearrange("a (p j) -> a p j", p=128), ivbf[:]
        )
        pbc = pp.tile([128, NTOK], f32, tag="pgen")
        for j in range(8):
            nc.tensor.matmul(
                pbc[:, 512 * j : 512 * (j + 1)],
                ones_row[:],
                ivrow[0:1, 512 * j : 512 * (j + 1)],
                start=True,
                stop=True,
            )

        # S = sum_n v_hat_n  (per C chunk)
        s_f = sm.tile([128, 2], f32, tag="sf")
        for cch in range(NCH):
            sjunk = prod.tile([128, NTOK], bf16, tag="sjunk", bufs=2)
            nc.vector.tensor_mul(sjunk[:], V[cch][:], pbc[:])
            nc.vector.tensor_reduce(
                s_f[:, cch : cch + 1], sjunk[:], AXL.X, AluOpType.add
            )
        s_bf = sm.tile([128, 2, 32], bf16, tag="sbf")
        for cch in range(NCH):
            nc.vector.tensor_copy(
                s_bf[:, cch, :],
                s_f[:, cch : cch + 1].broadcast_to((128, 32)),
            )

        # generation B: P.V plus the row dots r_x[n] = <X_n, S>
        pb = pp.tile([128, NTOK], f32, tag="pgen")
        for cch in range(NCH):
            dpv = prod.tile([128, NTOK], bf16, tag="prod")
            nc.vector.tensor_mul(dpv[:], P[cch][:], V[cch][:])
            matvec(pb, 0, ones32[:], dpv, cch)
            matvec(pb, 1, s_bf[:, cch, :], I[cch][:], cch)
            matvec(pb, 2, s_bf[:, cch, :], P[cch][:], cch)

        rb = rows.tile([128, NTOK], bf16, tag="rb")
        nc.scalar.copy(rb[0:96, :], pb[0:96, :])
        d_pv = row_tile(rb, 0)
        r_i = row_tile(rb, 32)
        r_p = row_tile(rb, 64)

        # normalized diagonal dots (into one [128,2,32] tile)
        dgl = sm.tile([128, 2, 32], f32, tag="dgl")
        nc.vector.tensor_mul(dgl[:, 0, :], d_iv[:], inv_i[:])
        nc.vector.tensor_mul(dgl[:, 0, :], dgl[:, 0, :], inv_v[:])
        nc.vector.tensor_mul(dgl[:, 1, :], d_pv[:], inv_p[:])
        nc.vector.tensor_mul(dgl[:, 1, :], dgl[:, 1, :], inv_v[:])
        nc.vector.tensor_reduce(
            pn_acc[:, 2 * b : 2 * b + 1], dgl[:, 0, :], AXL.X, AluOpType.add
        )
        nc.vector.tensor_reduce(
            pn_acc[:, 2 * b + 1 : 2 * b + 2], dgl[:, 1, :], AXL.X, AluOpType.add
        )

        # neg = (r_x * inv_x - diag) / (N-1); accumulate sigmoid(neg)
        ng = sm.tile([128, 2, 32], f32, tag="ng")
        nc.vector.tensor_mul(ng[:, 0, :], r_i[:], inv_i[:])
        nc.vector.tensor_mul(ng[:, 1, :], r_p[:], inv_p[:])
        nc.vector.tensor_sub(ng[:], ng[:], dgl[:])
        sg = sm.tile([128, 2, 32], f32, tag="sg")
        nc.scalar.activation(sg[:], ng[:], AFT.Exp, scale=-1.0 / (NTOK - 1))
        nc.vector.tensor_scalar_add(sg[:], sg[:], 1.0)
        sgr = sm.tile([128, 2, 32], f32, tag="sgr")
        nc.vector.reciprocal(sgr[:], sg[:])
        nc.vector.tensor_reduce(
            pn_acc[:, 4 + b : 5 + b], sgr[:], AXL.XY, AluOpType.add
        )

    # ------------------------------------------------------------------
    # Final partial outputs
    # ------------------------------------------------------------------
    pctx.close()
    with tc.tile_pool(name="pf", bufs=1, space="PSUM") as pf:
        fin_ps = pf.tile([1, 6], f32, tag="fin")
        nc.tensor.matmul(
            fin_ps[:], ones_f32[:], pn_acc[:], start=True, stop=True
        )
        fin = sm.tile([1, 6], f32, tag="fin")
        nc.vector.tensor_copy(fin[:], fin_ps[:])
        # pos values: mean of diag dots; nll = ln(sigmoid(pos)+eps), summed
        w = sm.tile([1, 16], f32, tag="w")
        nc.scalar.activation(w[:, 0:4], fin[:, 0:4], AFT.Exp, scale=-1.0 / NTOK)
        nc.vector.tensor_scalar_add(w[:, 0:4], w[:, 0:4], 1.0)
        nc.vector.reciprocal(w[:, 4:8], w[:, 0:4])
        nc.vector.tensor_scalar_add(w[:, 4:8], w[:, 4:8], EPS)
        nc.scalar.activation(w[:, 8:12], w[:, 4:8], AFT.Ln)
        nc.vector.tensor_reduce(out_sb[:, 0:1], w[:, 8:12], AXL.X, AluOpType.add)
        # neg sigmoid sum over both batches
        nc.vector.tensor_add(out_sb[:, 1:2], fin[:, 4:5], fin[:, 5:6])

    nc.sync.dma_start(out_d[:, :], out_sb[:])


@lru_cache(maxsize=1)
def _build_module():
    nc = bacc.Bacc("TRN2", target_bir_lowering=False, debug=False)
    f32 = dt.float32
    fimg = nc.dram_tensor("fimg", [BPC, C, NTOK], f32, kind="ExternalInput").ap()
    fpt = nc.dram_tensor("fpt", [BPC, C, NTOK], f32, kind="ExternalInput").ap()
    fbev = nc.dram_tensor("fbev", [BPC, C, NTOK], f32, kind="ExternalInput").ap()
    gall = nc.dram_tensor("gall", [B_FULL, GPACK], f32, kind="ExternalInput").ap()
    out_d = nc.dram_tensor("out", [1, 4], f32, kind="ExternalOutput").ap()
    with tile.TileContext(nc) as tc:
        with ExitStack() as ctx:
            _build_kernel(ctx, tc, fimg, fpt, fbev, gall, out_d)
    nc.compile()
    return nc


def _make_in_maps(
    image_global, point_global, text_global, image_features, point_features,
    bev_features, logit_scale,
):
    gi = np.ascontiguousarray(image_global, dtype=np.float32)
    gp = np.ascontiguousarray(point_global, dtype=np.float32)
    gt = np.ascontiguousarray(text_global, dtype=np.float32)
    fi = np.ascontiguousarray(image_features, dtype=np.float32).reshape(
        B_FULL, C, NTOK
    )
    fp = np.ascontiguousarray(point_features, dtype=np.float32).reshape(
        B_FULL, C, NTOK
    )
    fb = np.ascontiguousarray(bev_features, dtype=np.float32).reshape(
        B_FULL, C, NTOK
    )
    lsv = float(np.asarray(logit_scale, dtype=np.float32).reshape(()))
    gall = np.concatenate(
        [
            gi,
            gp,
            gt,
            np.eye(16, dtype=np.float32),
            np.full((16, 1), lsv, dtype=np.float32),
        ],
        axis=1,
    )
    assert gall.shape == (B_FULL, GPACK)
    in_maps = []
    for core in range(N_CORES):
        sl = slice(core * BPC, (core + 1) * BPC)
        in_maps.append(
            {
                "fimg": fi[sl],
                "fpt": fp[sl],
                "fbev": fb[sl],
                "gall": gall,
            }
        )
    return in_maps


def _combine(results):
    outs = np.stack([np.asarray(r["out"]).reshape(4) for r in results])
    pos_nll_sum = outs[:, 0].sum()
    neg_sig_sum = outs[:, 1].sum()
    l_global_raw = outs[0, 2]
    l_cross_raw = outs[0, 3]

    pos_loss = -pos_nll_sum / (2 * B_FULL)
    neg_loss = neg_sig_sum / (B_FULL * 2 * NTOK)
    l_local = 0.5 * (pos_loss + 0.1 * neg_loss)
    l_global = 1.0 * l_global_raw
    l_cross = 0.5 * l_cross_raw
    total = l_global + l_local + l_cross
    return np.array([l_global, l_local, l_cross, total], dtype=np.float32)


@lru_cache(maxsize=1)
def _build_null_module():
    """Trivial passthrough module, used to calibrate dispatch overhead."""
    nc = bacc.Bacc("TRN2", target_bir_lowering=False, debug=False)
    f32 = dt.float32
    xin = nc.dram_tensor("x", [1, 4], f32, kind="ExternalInput").ap()
    out_d = nc.dram_tensor("out", [1, 4], f32, kind="ExternalOutput").ap()
    with tile.TileContext(nc) as tc:
        with ExitStack() as ctx:
            p = ctx.enter_context(tc.tile_pool(name="t", bufs=1))
            t = p.tile([1, 4], f32)
            nc.sync.dma_start(t[:], xin[:, :])
            nc.sync.dma_start(out_d[:, :], t[:])
    nc.compile()
    return nc


_RUNNER_CACHE = {}


def _get_runner(key="main"):
    """Build (once) a persistent jitted shard_map callable for the module."""
    if key in _RUNNER_CACHE:
        return _RUNNER_CACHE[key]
    import jax
    from jax.experimental.shard_map import shard_map
    from jax.sharding import Mesh, PartitionSpec

    from concourse import bass2jax as b2j

    nc = _build_module() if key == "main" else _build_null_module()
    b2j.install_neuronx_cc_hook()
    partition_name = (
        nc.partition_id_tensor.name if nc.partition_id_tensor else None
    )
    in_names, out_names, out_avals, zero_outs = [], [], [], []
    for alloc in nc.m.functions[0].allocations:
        if not isinstance(alloc, mybir.MemoryLocationSet):
            continue
        name = alloc.memorylocations[0].name
        if alloc.kind == "ExternalInput":
            if name != partition_name:
                in_names.append(name)
        elif alloc.kind == "ExternalOutput":
            out_names.append(name)
            shape = tuple(alloc.tensor_shape)
            dtype = mybir.dt.np(alloc.dtype)
            out_avals.append(jax.core.ShapedArray(shape, dtype))
            zero_outs.append(np.zeros(shape, dtype))
    n_params = len(in_names)
    n_outs = len(out_avals)
    all_in = list(in_names) + list(out_names)
    if partition_name is not None:
        all_in.append(partition_name)

    def _body(*args):
        operands = list(args)
        if partition_name is not None:
            operands.append(b2j.partition_id_tensor())
        outs = b2j._bass_exec_p.bind(
            *operands,
            out_avals=tuple(out_avals),
            in_names=tuple(all_in),
            out_names=tuple(out_names),
            lowering_input_output_aliases=(),
            sim_require_finite=True,
            sim_require_nnan=True,
            nc=nc,
        )
        return tuple(outs)

    devices = jax.devices()[:N_CORES]
    mesh = Mesh(np.asarray(devices), ("core",))
    sharded = jax.jit(
        shard_map(
            _body,
            mesh=mesh,
            in_specs=(PartitionSpec("core"),) * (n_params + n_outs),
            out_specs=(PartitionSpec("core"),) * n_outs,
            check_rep=False,
        ),
        donate_argnums=tuple(range(n_params, n_params + n_outs)),
        keep_unused=True,
    )
    r = {
        "sharded": sharded,
        "in_names": in_names,
        "out_names": out_names,
        "out_avals": out_avals,
        "zero_outs": zero_outs,
        "mesh": mesh,
    }
    _RUNNER_CACHE[key] = r
    return r


def _concat_inputs(r, **inputs):
    in_maps = _make_in_maps(**inputs)
    per_core = [[np.asarray(m[n]) for n in r["in_names"]] for m in in_maps]
    return [
        np.concatenate([per_core[c][i] for c in range(N_CORES)], axis=0)
        for i in range(len(r["in_names"]))
    ]


def _concat_zeros(r):
    return [
        np.zeros((N_CORES * z.shape[0], *z.shape[1:]), z.dtype)
        for z in r["zero_outs"]
    ]


def _run_concat(r, concat_in):
    out_arrs = r["sharded"](*concat_in, *_concat_zeros(r))
    return [
        {
            name: np.asarray(out_arrs[i]).reshape(
                N_CORES, *r["out_avals"][i].shape
            )[c]
            for i, name in enumerate(r["out_names"])
        }
        for c in range(N_CORES)
    ]


def kernel(**inputs):
    r = _get_runner()
    results = _run_concat(r, _concat_inputs(r, **inputs))
    return _combine(results)
